# revision 47
# baseline (speedup 1.0000x reference)
"""AttentionBlock (GroupNorm + single-head self-attention + residual) as a
Bass/Tile kernel for one Trainium2 chip (8 NeuronCores), SPMD data-parallel.

Contract: kernel(**inputs) takes the FULL unsharded inputs of
reference.setup_inputs() and returns the FULL (4, 256, 64, 64) fp32 output.

Sharding: 4 images x 2 query-halves -> 8 cores. Each core receives the full
image x with its query half rotated to columns [0:2048] (attention and
GroupNorm are permutation-invariant over positions), computes GroupNorm
stats on-chip (bn_stats over 512-col chunks as the DMAs land + indicator
matmuls for the cross-partition group reduce), folds GN into the QKV
weights, builds K/Q (channels-major) and V transposed (positions-major, so
the PV matmul needs no on-chip transposes), runs flash-style attention over
512-column query chunks with transposed scores S^T = k^T q, projects, adds
bias + residual, and writes its (256, 2048) output slice. The host
rotates/scatters slices.

Default precision is "fp8": K/Q/V and the softmax weights are quantized to
e4m3 at psum-evacuation time (zero extra instructions), and every attention
matmul runs in DoubleRow perf mode — 2 fp8 contraction rows per PE cell per
cycle — so the 256-deep channel contraction of S^T and the paired j-tile
contraction of PV each collapse to ONE matmul (~2x PE throughput vs f32r).
exp runs with a -3 bias so the largest softmax weight stays under the e4m3
max of 448 (the e^-3 factor cancels in normalization). The softmax
denominator accumulates on the PE itself via a DoubleRow all-ones matmul
into a dedicated psum bank (the DVE add-tree the f32r path uses would gate
the faster fp8 pipeline). j-tiles are processed in pairs: both S^T halves
land in one [128,1024] 2-bank psum tile and a single exp instruction covers
the pair, halving ACT's ~293ns fixed per-instruction overhead. End-to-end
rel err ~8e-3 on HW (gate 2e-2); prec="f32r" is the accurate fallback.

The repeat-timing builds (unroll=2) duplicate the body with double-buffered
io/weight/KQV pools and segregated DMA rings (inputs on the SP HWDGE ring,
outputs on the ACT ring), so body i+1's x DMA + GroupNorm stats + folds all
stream during body i's attention — the measured steady-state is per-body
throughput. HAM warmer matmuls are only emitted for unroll=1 builds, where
the PE would otherwise idle through the load window.

This container's walrus build rejects any instruction carrying more than
one sync wait, while Tile freely attaches several; split_waits() below
rebuilds the basic blocks after scheduling, hoisting excess waits onto
single-wait NOPs inserted before the instruction on the same engine.
"""

import numpy as np

import bass_rust
import concourse.bass as bass
import concourse.mybir as mybir
import concourse.tile as tile
from concourse.bass import ts
from concourse.bass_utils import run_bass_kernel_spmd


# ---------------------------------------------------------------------------
# walrus single-sync-wait workaround


import bass_rust
import concourse.mybir as mybir

_counter = [0]


def _mk_nop(engine, wait):
    _counter[0] += 1
    nop = mybir.InstNoOp(name=f"WSPLIT-{_counter[0]}", ins=[], outs=[])
    nop.engine = engine
    nop.sync_info = bass_rust.SyncInfo(on_wait=[wait], on_update=[])
    return nop


def split_waits(nc, verbose=False):
    f = nc.m.functions[0]
    new_blocks = []
    n_split = 0
    for blk in f.blocks:
        insts = blk.instructions
        out = []
        for inst in insts:
            si = inst.sync_info
            if si is not None and si.on_wait and len(si.on_wait) > 1:
                waits = list(si.on_wait)
                for w in waits[1:]:
                    out.append(_mk_nop(inst.engine, w))
                si.on_wait = waits[:1]
                n_split += 1
            out.append(inst)
        new_blocks.append(bass_rust.BasicBlock(name=blk.name, instructions=out))
    f.blocks = new_blocks
    if verbose:
        print(f"split_waits: split {n_split} instructions")
    return n_split

# ---------------------------------------------------------------------------
# kernel builder + host-side sharding


import numpy as np

import concourse.bass as bass
import concourse.mybir as mybir
import concourse.tile as tile
from concourse.bass import ts

DT = mybir.dt.float32
DR = mybir.dt.float32r
AF = mybir.ActivationFunctionType
OP = mybir.AluOpType

C = 256
N = 4096
L = 2048
IC = 512  # i-chunk size
NCH = L // IC  # 4 chunks
NJT = N // 128  # 32 j-tiles
CT = C // 128  # 2 channel tiles
GROUPS = 8
EPS = 1e-5
SCALE = C ** -0.5


F8 = mybir.dt.float8e4
DRMODE = mybir.MatmulPerfMode.DoubleRow
PREC = "fp8"  # default precision mode for kernel()


def build(split=True, repeat=1, prec="fp8", no_in_dma=False,
          skip_attn=False, skip_dn=False, skip_exp=False, unroll=1):
    # "fp8": K/Q/V/P(softmax weights) quantized to e4m3; attention matmuls
    # run in DoubleRow perf mode (2 fp8 contraction rows per PE cell per
    # cycle, so the 256-deep channel contraction and paired j-tile
    # contractions each collapse to ONE matmul). "f32r": TF32-class path.
    DA = DR if prec == "f32r" else F8  # attention dtype
    nc = bass.Bass()

    x_d = nc.declare_dram_parameter("x", [CT, 128, N], DR, isOutput=False)
    wq_d = nc.declare_dram_parameter("wqT", [CT, 128, C], DR, isOutput=False)
    wk_d = nc.declare_dram_parameter("wkT", [CT, 128, C], DR, isOutput=False)
    wv_d = nc.declare_dram_parameter("wvT", [CT, 128, C], DR, isOutput=False)
    wp_d = nc.declare_dram_parameter("wpT", [CT, 128, C], DR, isOutput=False)
    # packed small params: per channel-tile [bq bk bv bp gnw gnb | G(8)] = 14
    bias6_d = nc.declare_dram_parameter("bias6", [CT, 128, 14], DT, isOutput=False)
    gt_d = nc.declare_dram_parameter("GT", [GROUPS, CT * 128], DT, isOutput=False)
    y_d = nc.declare_dram_parameter("y", [CT, 128, L], DT, isOutput=True)

    with tile.TileContext(nc) as tc:
        with (
            tc.tile_pool(name="io", bufs=unroll) as io,
            tc.tile_pool(name="wp_", bufs=unroll) as wpool,
            tc.tile_pool(name="kvq", bufs=unroll) as kvq,
            tc.tile_pool(name="pp", bufs=7) as ppool,
            tc.tile_pool(name="mis", bufs=3) as mis,
            tc.tile_pool(name="ps_s", bufs=2, space="PSUM") as ps_s,
            tc.tile_pool(name="ps_pv", bufs=1, space="PSUM") as ps_pv,
            tc.tile_pool(name="ps_dn", bufs=1, space="PSUM") as ps_dnp,
            tc.tile_pool(name="ps_m", bufs=1, space="PSUM") as ps_m,
        ):
            # ---------- loop-invariant constants (outside the repeat loop) ----
            ones_w = 2 * 128 if prec == "fp8" else 128
            ones_sq_f = wpool.tile([128, ones_w], DT, tag="ones_sq_f", name="ones_sq_f")
            nc.vector.memset(ones_sq_f[:], 1.0)
            ones_sq = wpool.tile([128, ones_w], DA, tag="ones_sq", name="ones_sq")
            if prec == "fp8":
                nc.scalar.copy(ones_sq[:], ones_sq_f[:])
            else:
                nc.vector.tensor_copy(ones_sq[:], ones_sq_f[:])
            eps_t = wpool.tile([GROUPS, 1], DT, tag="eps_t", name="eps_t")
            nc.vector.memset(eps_t[:], EPS)
            negb = wpool.tile([128, 1], DT, tag="negb", name="negb")
            nc.vector.memset(negb[:], -3.0)

            def body(_it=None):
                # ---------- loads ----------
                x_t = [io.tile([128, N], DR, tag=f"x{t}", name=f"x{t}") for t in range(CT)]
                w_in = {}
                for nm in ("q", "k", "v", "p"):
                    w_in[nm] = [io.tile([128, C], DR, tag=f"w{nm}{t}", name=f"w{nm}{t}") for t in range(CT)]
                b6_t = [io.tile([128, 14], DT, tag=f"b6{t}", name=f"b6{t}") for t in range(CT)]
                gt_t = io.tile([GROUPS, CT * 128], DT, tag="gt", name="gt")

                wd = {"q": wq_d, "k": wk_d, "v": wv_d, "p": wp_d}
                # ALL input DMAs ride the SP (sync) HWDGE ring; outputs ride
                # the ACT (scalar) ring. With the 2x-unrolled repeat loop this
                # keeps the next body's input prefetch from queueing behind
                # this body's output DMAs (rings are FIFO), so loads stream
                # during the previous body's attention. tiny params first; x
                # in 512-col chunks so bn_stats consumes them as they arrive;
                # weights last (only needed once stats complete).
                if no_in_dma:
                    # diagnostic: zero-init via memset, no input DMA traffic
                    for t in range(CT):
                        nc.vector.memset(b6_t[t][:], 0.01)
                        nc.vector.memset(x_t[t][:].bitcast(DT), 0.01)
                        for nm in ("q", "k", "v", "p"):
                            nc.vector.memset(w_in[nm][t][:].bitcast(DT), 0.01)
                    nc.vector.memset(gt_t[:], 0.01)
                else:
                    for t in range(CT):
                        nc.sync.dma_start(b6_t[t][:], bias6_d[t])
                    nc.sync.dma_start(gt_t[:], gt_d[:])
                    for a in range(8):
                        for t in range(CT):
                            nc.sync.dma_start(x_t[t][:, ts(a, 512)], x_d[t, :, ts(a, 512)])
                    for t in range(CT):
                        for nm in ("q", "k", "v", "p"):
                            nc.sync.dma_start(w_in[nm][t][:], wd[nm][t])

                b_in = {nm: [b6_t[t][:, i:i + 1] for t in range(CT)]
                        for i, nm in enumerate(("q", "k", "v", "p", "gw", "gb"))}
                g_t = [b6_t[t][:, 6:14] for t in range(CT)]

                # HAM warmers: dummy matmuls keyed to each arriving x chunk keep
                # the PE's activity monitor at full clock through the DMA/stats
                # window, so the real matmul stream starts warm (2.4 GHz). Only
                # for non-unrolled builds — in the 2x-unrolled pipeline the PE
                # is busy with the previous body's attention during the loads.
                if unroll == 1:
                    for a in range(8):
                        for t in range(CT):
                            ps_w = ps_m.tile([128, 512], DT, tag="ps_m", name="ps_warm")
                            nc.tensor.matmul(
                                ps_w[:], x_t[t][:, a * 512: a * 512 + 128],
                                x_t[t][:, ts(a, 512)],
                                start=True, stop=True,
                            )

                # ---------- GroupNorm stats (bn_stats per arriving chunk) ----
                parts = [wpool.tile([128, 2], DT, tag=f"parts{t}", name=f"parts{t}") for t in range(CT)]
                bns_t = [wpool.tile([128, 8 * 6], DT, tag=f"bns{t}", name=f"bns{t}") for t in range(CT)]
                for a in range(8):  # interleaved: stats track chunk arrivals
                    for t in range(CT):
                        nc.vector.bn_stats(
                            bns_t[t][:, a * 6:(a + 1) * 6],
                            x_t[t][:, ts(a, 512)].bitcast(DT),
                        )
                for t in range(CT):
                    mv = wpool.tile([128, 2], DT, tag="mv", name=f"mv{t}")
                    nc.vector.bn_aggr(mv[:], bns_t[t][:].rearrange("p (a s) -> p a s", s=6))
                    # parts = [mean_c, ex2_c = var_c + mean_c^2]
                    nc.vector.tensor_mul(parts[t][:, 1:2], mv[:, 0:1], mv[:, 0:1])
                    nc.vector.tensor_add(parts[t][:, 1:2], parts[t][:, 1:2], mv[:, 1:2])
                    nc.vector.tensor_copy(parts[t][:, 0:1], mv[:, 0:1])

                # group means via indicator matmul (fp32): (8,2) = 32*[mean_g, ex2_g]
                ps_g = ps_m.tile([GROUPS, 2], DT, tag="ps_m", name="ps_g")
                for t in range(CT):
                    nc.tensor.matmul(
                        ps_g[:], g_t[t], parts[t][:],
                        start=(t == 0), stop=(t == CT - 1),
                    )
                st_mv = wpool.tile([GROUPS, 2], DT, tag="st_mv", name="st_mv")
                nc.vector.tensor_scalar_mul(st_mv[:], ps_g[:], 1.0 / 32)
                st_var = wpool.tile([GROUPS, 1], DT, tag="st_var", name="st_var")
                nc.vector.tensor_mul(st_var[:], st_mv[:, 0:1], st_mv[:, 0:1])
                nc.vector.tensor_sub(st_var[:], st_mv[:, 1:2], st_var[:])
                st_sd = wpool.tile([GROUPS, 1], DT, tag="st_sd", name="st_sd")
                nc.scalar.activation(st_sd[:], st_var[:], AF.Sqrt, bias=eps_t[:])
                st2 = wpool.tile([GROUPS, 2], DT, tag="st2", name="st2")
                nc.vector.tensor_copy(st2[:, 0:1], st_mv[:, 0:1])
                nc.vector.reciprocal(st2[:, 1:2], st_sd[:])

                # broadcast to channels (fp32 matmul): psum (128,2) = GT^T @ st2
                scale_c = [wpool.tile([128, 1], DT, tag=f"scale_c{t}", name=f"scale_c{t}") for t in range(CT)]
                bias_c = [wpool.tile([128, 1], DT, tag=f"bias_c{t}", name=f"bias_c{t}") for t in range(CT)]
                for t in range(CT):
                    ps_bc = ps_m.tile([128, 2], DT, tag="ps_m", name="ps_bc")
                    nc.tensor.matmul(ps_bc[:], gt_t[:, ts(t, 128)], st2[:], start=True, stop=True)
                    nc.vector.tensor_mul(scale_c[t][:], b_in["gw"][t], ps_bc[:, 1:2])
                    nc.vector.tensor_mul(bias_c[t][:], ps_bc[:, 0:1], scale_c[t][:])
                    nc.vector.tensor_sub(bias_c[t][:], b_in["gb"][t], bias_c[t][:])

                # ---------- fold GN into weights/biases ----------
                w_s = {}
                for nm in ("q", "k", "v"):
                    w_s[nm] = []
                    for t in range(CT):
                        wt = wpool.tile([128, C], DR, tag=f"ws{nm}{t}", name=f"ws{nm}{t}")
                        nc.vector.tensor_scalar_mul(
                            wt[:], w_in[nm][t][:].bitcast(DT), scale_c[t][:]
                        )
                        w_s[nm].append(wt)
                b_f = {}
                for nm in ("q", "v"):
                    b_f[nm] = []
                    for ot in range(CT):
                        ps_f = ps_m.tile([128, 1], DT, tag="ps_m", name="ps_f")
                        for ct in range(CT):
                            nc.tensor.matmul(
                                ps_f[:], w_in[nm][ct][:, ts(ot, 128)].bitcast(DT),
                                bias_c[ct][:],
                                start=(ct == 0), stop=(ct == CT - 1),
                            )
                        bf = wpool.tile([128, 1], DT, tag=f"bf{nm}{ot}", name=f"bf{nm}{ot}")
                        nc.vector.tensor_add(bf[:], b_in[nm][ot], ps_f[:])
                        b_f[nm].append(bf)
                b_f["p"] = []
                for ot in range(CT):
                    ps_f2 = ps_m.tile([128, 1], DT, tag="ps_m", name="ps_f2")
                    for ct in range(CT):
                        nc.tensor.matmul(
                            ps_f2[:], w_in["p"][ct][:, ts(ot, 128)].bitcast(DT),
                            b_f["v"][ct][:],
                            start=(ct == 0), stop=(ct == CT - 1),
                        )
                    bf = wpool.tile([128, 1], DT, tag=f"bfp{ot}", name=f"bfp{ot}")
                    nc.vector.tensor_add(bf[:], b_in["p"][ot], ps_f2[:])
                    b_f["p"].append(bf)

                # ---------- K, Q (regular layout), V^T ----------
                # fp8: K and Q live in single concatenated tiles so a
                # rearranged [128, 2(ct), .] view feeds DoubleRow matmuls
                # (contraction 256 = both channel tiles in one MM). k_t/q_t
                # below are per-ct AP views into them either way.
                if prec == "fp8":
                    k2 = kvq.tile([128, CT * N], DA, tag="k2", name="k2")
                    q2 = kvq.tile([128, CT * L], DA, tag="q2", name="q2")
                    k_t = [k2[:, ts(t, N)] for t in range(CT)]
                    q_t = [q2[:, ts(t, L)] for t in range(CT)]
                else:
                    k_t = [kvq.tile([128, N], DA, tag=f"k{t}", name=f"k{t}")[:] for t in range(CT)]
                    q_t = [kvq.tile([128, L], DA, tag=f"q{t}", name=f"q{t}")[:] for t in range(CT)]
                vt_big = kvq.tile([128, NJT * C], DA, tag="vt_big", name="vt_big")

                def evac_bias(dst, ps, bias_ap, use_dve):
                    # alternate psum evacuation between ACT and DVE so neither
                    # serializes the PE's psum-slot recycling
                    if use_dve:
                        nc.vector.tensor_scalar_add(dst, ps, bias_ap)
                    else:
                        nc.scalar.activation(dst, ps, AF.Identity, bias=bias_ap)

                # paired 1024-col psum tiles (2 banks): halves filled by separate
                # matmuls (psum-bank limit is 512 fp32 per matmul), evacuated by
                # ONE ACT/DVE instruction — halves the per-instruction fixed
                # overhead (~293ns ACT, ~157ns DVE) on the evac stream.
                def mm_kq(nm, ps_half, ot, jc):
                    for ct in range(CT):
                        nc.tensor.matmul(
                            ps_half, w_s[nm][ct][:, ts(ot, 128)],
                            x_t[ct][:, ts(jc, 512)],
                            start=(ct == 0), stop=(ct == CT - 1),
                        )

                ev = 0
                for ot in range(CT):
                    for jcp in range(N // 1024):
                        ps_k = ps_s.tile([128, 1024], DT, tag="ps_s", name="ps_k")
                        for h in range(2):
                            mm_kq("k", ps_k[:, ts(h, 512)], ot, 2 * jcp + h)
                        # no K bias: bk^T q is constant over the softmax axis,
                        # so it cancels exactly in the normalization
                        if ev % 2:
                            nc.vector.tensor_copy(k_t[ot][:, ts(jcp, 1024)], ps_k[:])
                        else:
                            nc.scalar.copy(k_t[ot][:, ts(jcp, 1024)], ps_k[:])
                        ev += 1
                for ot in range(CT):
                    for icp in range(L // 1024):
                        ps_q = ps_s.tile([128, 1024], DT, tag="ps_s", name="ps_q")
                        for h in range(2):
                            mm_kq("q", ps_q[:, ts(h, 512)], ot, 2 * icp + h)
                        evac_bias(q_t[ot][:, ts(icp, 1024)], ps_q[:],
                                  b_f["q"][ot][:], ev % 2)
                        ev += 1
                for vp in range(NJT // 4):
                    ps_v = ps_s.tile([128, 1024], DT, tag="ps_s", name="ps_v")
                    for h in range(4):
                        jt = 4 * vp + h
                        for ct in range(CT):
                            nc.tensor.matmul(
                                ps_v[:, ts(h, C)], x_t[ct][:, ts(jt, 128)],
                                w_s["v"][ct][:],
                                start=(ct == 0), stop=(ct == CT - 1),
                            )
                    if vp % 2:
                        nc.vector.tensor_copy(vt_big[:, ts(vp, 1024)], ps_v[:])
                    else:
                        nc.scalar.copy(vt_big[:, ts(vp, 1024)], ps_v[:])

                # ---------- attention over i-chunks ----------
                # j-tiles processed in PAIRS: S^T for jt=2a,2a+1 lands in one
                # [128,1024] 2-bank psum tile and ONE exp instruction covers
                # both (halves ACT fixed overhead).
                # fp8: S^T is one DoubleRow matmul per jt (contraction 256 =
                # both channel tiles via the k2/q2 pair views), PV is one
                # DoubleRow matmul per (pair, ct) contracting both j-tiles of
                # the pair at once, and the softmax denominator accumulates on
                # the PE via a DoubleRow all-ones matmul into its own bank.
                # f32r: plain matmuls; denominator via DVE add tree (off the
                # PE, which is the bottleneck in this mode).
                NJP = NJT // 2  # 16 pairs
                if skip_exp:
                    pt0 = ppool.tile([128, 1024], DA, tag="pt", name="pt0")
                    nc.vector.memset(pt0[:].bitcast(DT), 0.01)
                if prec == "fp8":
                    k2r = k2[:].rearrange("p (t j) -> p t j", t=CT)
                    q2r = q2[:].rearrange("p (t i) -> p t i", t=CT)
                    vtr = vt_big[:].rearrange("p (j c) -> p j c", c=C)
                    onesr = ones_sq[:].rearrange("p (t j) -> p t j", t=2)
                for ic in range(NCH):
                    ps_pv_t = [ps_pv.tile([128, IC], DT, tag=f"pspv{ct}", name=f"pspv{ct}") for ct in range(CT)]
                    if prec == "fp8":
                        ps_dn = ps_dnp.tile([128, IC], DT, tag="dn", name="ps_dn")
                    else:
                        acc = mis.tile([128, 1024], DT, tag="acc", name="acc")

                    # software-pipelined: emit S^T[a+1] before PV[a] so PE
                    # always has score work queued while exp[a] completes.
                    def emit_s(a):
                        ps_sc = ps_s.tile([128, 1024], DT, tag="ps_s", name="ps_sc")
                        for h in range(2):
                            jt = 2 * a + h
                            if prec == "fp8":
                                nc.tensor.matmul(
                                    ps_sc[:, ts(h, 512)],
                                    k2r[:, :, ts(jt, 128)],
                                    q2r[:, :, ts(ic, IC)],
                                    start=True, stop=True, perf_mode=DRMODE,
                                )
                            else:
                                for ct in range(CT):
                                    nc.tensor.matmul(
                                        ps_sc[:, ts(h, 512)], k_t[ct][:, ts(jt, 128)],
                                        q_t[ct][:, ts(ic, IC)],
                                        start=(ct == 0), stop=(ct == CT - 1),
                                    )
                        if skip_exp:
                            return pt0
                        pt = ppool.tile([128, 1024], DA, tag="pt", name="pt")
                        if prec == "fp8":
                            # exp(S*scale - 3): keeps weights under fp8e4 max
                            # (448); the e^-3 factor cancels in normalization
                            nc.scalar.activation(pt[:], ps_sc[:], AF.Exp,
                                                 scale=SCALE, bias=negb[:])
                        else:
                            nc.scalar.activation(pt[:], ps_sc[:], AF.Exp, scale=SCALE)
                        return pt

                    def emit_pv(a, pt):
                        if prec == "fp8":
                            ptr = pt[:].rearrange("p (t i) -> p t i", t=2)
                            if not skip_dn:
                                nc.tensor.matmul(
                                    ps_dn[:], onesr, ptr,
                                    start=(a == 0), stop=(a == NJP - 1),
                                    perf_mode=DRMODE,
                                )
                            for ct in range(CT):
                                nc.tensor.matmul(
                                    ps_pv_t[ct][:],
                                    vtr[:, 2 * a:2 * a + 2, ct * 128:(ct + 1) * 128],
                                    ptr,
                                    start=(a == 0), stop=(a == NJP - 1),
                                    perf_mode=DRMODE,
                                )
                            return
                        if a == 0:
                            nc.vector.tensor_copy(acc[:], pt[:].bitcast(DT))
                        else:
                            nc.vector.tensor_add(acc[:], acc[:], pt[:].bitcast(DT))
                        for h in range(2):
                            jt = 2 * a + h
                            for ct in range(CT):
                                nc.tensor.matmul(
                                    ps_pv_t[ct][:],
                                    vt_big[:, jt * C + ct * 128: jt * C + ct * 128 + 128],
                                    pt[:, ts(h, 512)],
                                    start=(jt == 0), stop=(jt == NJT - 1),
                                )

                    if skip_attn:
                        # diagnostic: no S/exp/PV work; one tiny matmul per
                        # psum tile so downstream reads have a writer
                        for ct in range(CT):
                            nc.tensor.matmul(
                                ps_pv_t[ct][:], ones_sq[:, 0:128],
                                k_t[0][:, 0:IC], start=True, stop=True,
                            )
                    else:
                        pt_prev = emit_s(0)
                        for a in range(1, NJP):
                            pt_cur = emit_s(a)
                            emit_pv(a - 1, pt_prev)
                            pt_prev = pt_cur
                        emit_pv(NJP - 1, pt_prev)
                    rb_sb = mis.tile([128, IC], DT, tag="rb_sb", name="rb_sb")
                    if skip_attn or skip_dn:
                        nc.vector.memset(rb_sb[:], 1.0)
                    elif prec == "fp8":
                        nc.vector.reciprocal(rb_sb[:], ps_dn[:])
                    else:
                        # fold the two pair-halves then reduce+broadcast via
                        # the all-ones matmul
                        acch = mis.tile([128, IC], DT, tag="acch", name="acch")
                        nc.vector.tensor_add(acch[:], acc[:, 0:IC], acc[:, IC:1024])
                        acc_r = mis.tile([128, IC], DA, tag="acc_r", name="acc_r")
                        nc.vector.tensor_copy(acc_r[:], acch[:])
                        ps_db = ps_m.tile([128, IC], DT, tag="ps_m", name="ps_db")
                        nc.tensor.matmul(ps_db[:], ones_sq[:], acc_r[:], start=True, stop=True)
                        nc.vector.reciprocal(rb_sb[:], ps_db[:])
                    ao = [mis.tile([128, IC], DR, tag=f"ao{ct}", name=f"ao{ct}") for ct in range(CT)]
                    for ct in range(CT):
                        nc.vector.tensor_mul(ao[ct][:], ps_pv_t[ct][:], rb_sb[:])
                    for ot in range(CT):
                        # ps_y reuses the (now-read) ps_pv bank for this ot
                        ps_y = ps_pv.tile([128, IC], DT, tag=f"pspv{ot}", name="ps_y")
                        for ct in range(CT):
                            nc.tensor.matmul(
                                ps_y[:], w_in["p"][ct][:, ts(ot, 128)], ao[ct][:],
                                start=(ct == 0), stop=(ct == CT - 1),
                            )
                        y_sb = mis.tile([128, IC], DT, tag="y_sb", name="y_sb")
                        nc.vector.scalar_tensor_tensor(
                            y_sb[:], ps_y[:], b_f["p"][ot][:],
                            x_t[ot][:, ts(ic, IC)].bitcast(DT),
                            op0=OP.add, op1=OP.add,
                        )
                        nc.scalar.dma_start(y_d[ot, :, ts(ic, IC)], y_sb[:])

            if repeat == 1:
                for _u in range(unroll):
                    body()
            else:
                # hint all busy engines so the back-edge IRAM fetch prefetches
                # (bodies exceed one 16KiB IRAM block per engine)
                hints = (mybir.EngineType.PE, mybir.EngineType.Activation,
                         mybir.EngineType.DVE, mybir.EngineType.SP)
                with tc.For_i(0, repeat, 1, hint_engines=hints) as it:
                    for _u in range(unroll):
                        body(it)

    if split:
        split_waits(nc)
    return nc


# ---------------- host-side sharding helpers ----------------

def make_in_maps(inputs):
    x = np.asarray(inputs["x"], dtype=np.float32)
    n = x.shape[0]

    def wt(name):
        w = np.asarray(inputs[name], dtype=np.float32)
        return np.ascontiguousarray(w.T.reshape(CT, 128, C))

    wqT, wkT, wvT, wpT = wt("wq"), wt("wk"), wt("wv"), wt("wp")

    bias6 = np.zeros((CT, 128, 14), dtype=np.float32)
    for i, nm in enumerate(("bq", "bk", "bv", "bp", "gn_w", "gn_b")):
        bias6[:, :, i] = np.asarray(inputs[nm], np.float32).reshape(CT, 128)
    for t in range(CT):
        for p in range(128):
            bias6[t, p, 6 + (t * 128 + p) // 32] = 1.0  # G indicator
    GT = np.zeros((GROUPS, CT * 128), dtype=np.float32)
    for c in range(C):
        GT[c // 32, c] = 1.0

    in_maps = []
    for core in range(2 * n):
        b, h = divmod(core, 2)
        xb = x[b].reshape(CT, 128, N)
        if h == 0:
            xp = np.ascontiguousarray(xb)
        else:
            xp = np.ascontiguousarray(
                np.concatenate([xb[:, :, L:], xb[:, :, :L]], axis=2)
            )
        in_maps.append({
            "x": xp,
            "wqT": wqT, "wkT": wkT, "wvT": wvT, "wpT": wpT,
            "bias6": bias6, "GT": GT,
        })
    return in_maps


def assemble(results, n=4):
    out = np.zeros((n, C, 64, 64), dtype=np.float32)
    flat = out.reshape(n, C, N)
    for core, res in enumerate(results):
        b, h = divmod(core, 2)
        flat[b, :, h * L:(h + 1) * L] = res["y"].reshape(C, L)
    return out


_CACHE = {}


def kernel(**inputs) -> np.ndarray:
    n = np.asarray(inputs["x"]).shape[0]
    n_cores = 2 * n
    if "nc" not in _CACHE:
        _CACHE["nc"] = build(split=True, repeat=1, prec=PREC)
    nc = _CACHE["nc"]
    in_maps = make_in_maps(inputs)
    last_err = None
    for _attempt in range(2):  # one retry on transient axon/RPC failures
        try:
            res = run_bass_kernel_spmd(nc, in_maps, list(range(n_cores)))
            return assemble(res.results, n=n)
        except Exception as e:  # noqa: BLE001
            last_err = e
    raise last_err

